# revision 17
# baseline (speedup 1.0000x reference)
"""FFD sparse-matmul kernel for Trainium2 (8 NeuronCores).

Problem: out[b, r, d] = sum_i 1[rows_i == r] * vals_i * (x[b, cols_i, d]*scale[d] - offset[d])
  = (A @ xs')[b, r, d]   with xs' = x*scale - offset, A the static
[200000, 4096] sparse FFD matrix (12.8M nnz) densified.

Strategy (v3): make the BIG matrix the *moving* operand of the PE.
The baseline kept A stationary, paying a 128-column LDWEIGHTS (~107ns)
per 6 moving columns. Here the tiny control-point matrix xs' is the
stationary operand ([128, 12] per K-chunk, ~10ns loads) and A.T
streams through the array at 1 column/cycle.

Two column-group tiles (tile_position (0,0) and (0,32)) process two
512-row tiles concurrently in disjoint 32-column strips of the PE
array, nearly halving array time; the kernel is then HBM-bound
(~102.8MB/core of fp8 weights at ~358GB/s).

A is stored as fp8-e3m4 (1 byte/elem, HWDGE plain DMA - no SWDGE cast,
so SBUF write traffic stays 1B/elem), streamed on both HWDGE queues
(sync+scalar) in 2MB tiles. xs' is split hi/lo into two fp8 e3m4
stationary column groups (lo pre-scaled by 64 to clear the e3m4
subnormal floor); the host recombines out = hi + lo/64, restoring
fp16-class precision for xs with zero on-chip combine cost.

Row-sharded over 8 cores (25000 rows each), fp32 PSUM accumulation
over the 32 K-chunks, one scalar-engine PSUM->SBUF copy per 512-row
tile, output DMA chunked and overlapped with compute.
"""

import os
import numpy as np
import ml_dtypes

N_PTS = 200000
N_CTRL = 4096
B = 2
N_CORES = 8
ROWS_PER_CORE = N_PTS // N_CORES  # 25000
RT = 512                          # moving-tile rows (PE free dim, PSUM bank)
N_RT = -(-ROWS_PER_CORE // RT)    # 49 row tiles -> 25088 padded rows
R_PAD = N_RT * RT
KC = 128                          # contraction chunk (PE partition dim)
N_KC = N_CTRL // KC               # 32
FN = B * 3                        # 6 output columns (j = b*3 + d)
M2 = 2 * FN                       # 12 stationary columns: xs_hi | 64*xs_lo
LO_SCALE = 64.0
N_PAIR = N_RT // 2                # 24 col-group pairs (+1 leftover tile)
N_SLOT = N_RT - N_PAIR            # 25 slots per partition group in obuf
OCHUNK = 5                        # output-DMA granularity (slots per chunk)

F8 = ml_dtypes.float8_e3m4

LAST_RESULTS = None  # BassKernelResults of the most recent device run

_static_cache = {}  # fingerprint -> list of per-core moving-weight arrays
_nc_cache = {}


def _fingerprint(*arrays):
    h = 0
    for a in arrays:
        s = a[:: max(1, a.size // 4096)].tobytes()
        h ^= hash((a.size, s, float(a.astype(np.float64).sum())))
    return h


def _install_profile_shim():
    """Make trace=True work in images whose antenv lacks axon_hooks, and
    neuter the bucket artifact upload. Best-effort; harmless if partial."""
    import sys
    import types

    try:
        import concourse.bass_utils as bu

        bu.upload_artifacts = lambda tmpdir: f"local:{tmpdir}"
    except Exception:
        pass
    try:
        import antenv.axon_hooks  # noqa: F401

        return
    except ImportError:
        pass
    try:
        mod = types.ModuleType("antenv.axon_hooks")
        mod._hook = None
        mod.set_axon_ntff_profile_hook = lambda h: setattr(mod, "_hook", h)
        mod.get_axon_ntff_profile_hook = lambda: mod._hook
        sys.modules["antenv.axon_hooks"] = mod
        import antenv

        antenv.axon_hooks = mod
        if "/root/.axon_site/trn_agent_boot" not in sys.path:
            sys.path.insert(0, "/root/.axon_site/trn_agent_boot")
        from trn_boot import _ntff_profile_via_ctypes

        hook = _ntff_profile_via_ctypes("/opt/axon/libaxon_pjrt.so")
        if hook is not None:
            mod._hook = hook
    except Exception:
        pass


def _build_nc():
    import concourse.mybir as mybir
    from concourse import bacc
    from concourse.tile import TileContext

    f8, f32 = mybir.dt.float8e3, mybir.dt.float32
    nc = bacc.Bacc()
    # Pair-fused weights: one 4MB DMA per column-group pair, 32KB
    # contiguous per partition line (long descriptors amortize per-packet
    # HBM latency); plus one solo tile for the odd leftover row tile.
    wP = nc.declare_dram_parameter(
        "wP", [N_PAIR, KC, 2 * N_KC * RT], f8, isOutput=False
    )
    wS = nc.declare_dram_parameter("wS", [KC, N_KC * RT], f8, isOutput=False)
    xs = nc.declare_dram_parameter("xs", [KC, N_KC * M2], f8, isOutput=False)
    # out rows 0-11: even row-tiles (col group 0); rows 32-43: odd (group 1)
    out = nc.declare_dram_parameter("out", [44, N_SLOT * RT], f32, isOutput=True)

    w_q = [nc.sync, nc.scalar]  # alternate weight DMAs across both HWDGE queues

    with TileContext(nc) as tc:
        with (
            tc.tile_pool(name="wp", bufs=5) as wp,
            tc.tile_pool(name="cp", bufs=1) as cp,
            tc.tile_pool(name="op", bufs=2) as op,
            tc.tile_pool(name="pp", bufs=6, space="PSUM") as pp,
        ):
            xs_sb = cp.tile([KC, N_KC * M2], f8, tag="xs")
            nc.scalar.dma_start(out=xs_sb[:], in_=xs[:])

            HALF = N_KC * RT  # 16384 cols: a-tile in [0, HALF), b in [HALF, 2*HALF)
            obuf = None
            for t in range(N_SLOT):
                have_b = 2 * t + 1 < N_RT
                if have_b:
                    w_pair = wp.tile([KC, 2 * HALF], f8, tag="w")
                    if t == 0:
                        # Split the first pair into eighths, interleaved so
                        # the first matmuls start after ~0.5MB lands (subtile
                        # deps): a0, b0, a1, b1, a2, b2, a3, b3.
                        for i in range(8):
                            half, q = i % 2, i // 2
                            cs = slice(
                                half * HALF + q * HALF // 4,
                                half * HALF + (q + 1) * HALF // 4,
                            )
                            w_q[i % 2].dma_start(out=w_pair[:, cs], in_=wP[0][:, cs])
                    elif t == N_SLOT - 2:
                        # Quarter-split the last pair so its matmuls overlap
                        # the stream and the post-DMA tail shrinks.
                        for i, q in enumerate((0, 2, 1, 3)):
                            cs = slice(q * HALF // 2, (q + 1) * HALF // 2)
                            w_q[i % 2].dma_start(out=w_pair[:, cs], in_=wP[t][:, cs])
                    else:
                        w_q[t % 2].dma_start(out=w_pair[:], in_=wP[t])
                else:
                    w_pair = wp.tile([KC, N_KC * RT], f8, tag="w")
                    for i in range(2):  # halve the solo tile likewise
                        cs = slice(i * HALF // 2, (i + 1) * HALF // 2)
                        w_q[i % 2].dma_start(out=w_pair[:, cs], in_=wS[:, cs])
                ps = pp.tile([44, RT], f32, tag="ps")
                # Interleave the two column-group streams so consecutive
                # matmuls land in disjoint 32-column strips of the PE array
                # and run concurrently (~2x array throughput).
                for kc in range(N_KC):
                    nc.tensor.matmul(
                        ps[0:M2, :],
                        xs_sb[:, kc * M2 : (kc + 1) * M2],
                        w_pair[:, kc * RT : (kc + 1) * RT],
                        start=(kc == 0),
                        stop=(kc == N_KC - 1),
                        tile_position=(0, 0),
                    )
                    if have_b:
                        nc.tensor.matmul(
                            ps[32 : 32 + M2, :],
                            xs_sb[:, kc * M2 : (kc + 1) * M2],
                            w_pair[:, HALF + kc * RT : HALF + (kc + 1) * RT],
                            start=(kc == 0),
                            stop=(kc == N_KC - 1),
                            tile_position=(0, 32),
                        )
                if t % OCHUNK == 0:
                    obuf = op.tile([44, OCHUNK * RT], f32, tag="ob")
                osl = obuf[:, (t % OCHUNK) * RT : (t % OCHUNK + 1) * RT]
                nc.scalar.copy(out=osl[0:M2, :], in_=ps[0:M2, :])
                if have_b:
                    nc.vector.tensor_copy(
                        out=osl[32 : 32 + M2, :], in_=ps[32 : 32 + M2, :]
                    )
                if t % OCHUNK == OCHUNK - 1 or t == N_SLOT - 1:
                    blk = t - t % OCHUNK  # first slot of this block
                    n = t % OCHUNK + 1
                    dsl = slice(blk * RT, (blk + n) * RT)
                    nc.sync.dma_start(
                        out=out[0:M2, dsl], in_=obuf[0:M2, : n * RT]
                    )
                    n_b = min(blk + n, N_PAIR) - blk  # odd group: one fewer
                    if n_b > 0:
                        dsl_b = slice(blk * RT, (blk + n_b) * RT)
                        nc.scalar.dma_start(
                            out=out[32 : 32 + M2, dsl_b],
                            in_=obuf[32 : 32 + M2, : n_b * RT],
                        )
    nc.finalize()
    return nc


def _prepare_static(ffd_vals, ffd_rows, ffd_cols):
    """Densify the static sparse matrix into per-core fp8 moving tiles.

    Layout per core: w[rt, p, kc*RT + n] = A[rt*RT + n, kc*KC + p]
    (rt-th 512-row tile, kc-th 128-ctrl chunk, transposed so the
    contraction dim is the partition dim)."""
    key = _fingerprint(ffd_vals, ffd_rows, ffd_cols)
    if key in _static_cache:
        return _static_cache[key]

    try:
        from scipy.sparse import coo_matrix

        A = np.asarray(
            coo_matrix(
                (ffd_vals, (ffd_rows, ffd_cols)), shape=(N_PTS, N_CTRL)
            ).todense(),
            dtype=np.float32,
        )
    except Exception:
        A = np.zeros((N_PTS, N_CTRL), np.float32)
        np.add.at(A, (ffd_rows, ffd_cols), ffd_vals)

    wTs = []
    for c in range(N_CORES):
        Ac = A[c * ROWS_PER_CORE : (c + 1) * ROWS_PER_CORE]
        Ap = np.zeros((R_PAD, N_CTRL), F8)
        Ap[:ROWS_PER_CORE] = Ac.astype(F8)
        w = (
            Ap.view(np.uint8)
            .reshape(N_RT, RT, N_KC, KC)
            .transpose(0, 3, 2, 1)  # [rt, p, kc, n]
        )
        w = np.ascontiguousarray(w).reshape(N_RT, KC, N_KC * RT)
        # Pair-fuse: wP[t, p, :] = [tile 2t | tile 2t+1] per partition line.
        wp = np.concatenate([w[0:-1:2], w[1::2]], axis=2)  # [24, KC, 2*HALF]
        wTs.append((np.ascontiguousarray(wp).view(F8), w[-1].copy().view(F8)))

    del A
    _static_cache.clear()
    _static_cache[key] = wTs
    return wTs


def kernel(x, scale_vec, offset, ffd_vals, ffd_rows, ffd_cols):
    global LAST_RESULTS
    from concourse.bass_utils import run_bass_kernel_spmd

    x = np.asarray(x, np.float32)
    scale_vec = np.asarray(scale_vec, np.float32)
    offset = np.asarray(offset, np.float32)
    ffd_vals = np.asarray(ffd_vals, np.float32)
    ffd_rows = np.asarray(ffd_rows, np.int32)
    ffd_cols = np.asarray(ffd_cols, np.int32)

    wTs = _prepare_static(ffd_vals, ffd_rows, ffd_cols)

    # Dynamic (per-call) host prep: tiny. xs6[c, j] with j = b*3 + d.
    xs6 = (x * scale_vec[None, None, :] - offset[None, None, :]).transpose(
        1, 0, 2
    ).reshape(N_CTRL, FN).astype(np.float32)
    hi = xs6.astype(F8)
    lo = ((xs6 - hi.astype(np.float32)) * LO_SCALE).astype(F8)
    # Device layout: xs_t[p, kc*M2 + j] = hi[kc*KC + p, j], lo in cols 6-11.
    xs_t = np.zeros((KC, N_KC * M2), F8)
    xs_t_v = xs_t.view(np.uint8).reshape(KC, N_KC, M2)
    xs_t_v[:, :, :FN] = hi.view(np.uint8).reshape(N_KC, KC, FN).transpose(1, 0, 2)
    xs_t_v[:, :, FN:] = lo.view(np.uint8).reshape(N_KC, KC, FN).transpose(1, 0, 2)

    in_maps = [
        {"wP": wTs[c][0], "wS": wTs[c][1], "xs": xs_t} for c in range(N_CORES)
    ]

    if "nc" not in _nc_cache:
        _nc_cache["nc"] = _build_nc()
    nc = _nc_cache["nc"]

    trace = bool(os.environ.get("BASS_TRACE"))
    if trace:
        _install_profile_shim()
    try:
        res = run_bass_kernel_spmd(nc, in_maps, list(range(N_CORES)), trace=trace)
    except Exception:
        if not trace:
            raise
        os.environ.pop("BASS_TRACE", None)
        res = run_bass_kernel_spmd(nc, in_maps, list(range(N_CORES)), trace=False)
    LAST_RESULTS = res

    shards = []
    for c in range(N_CORES):
        o = np.asarray(res.results[c]["out"], np.float32)  # [44, N_SLOT*RT]
        o6 = np.empty((FN, R_PAD), np.float32)
        ev = o[0:FN] + o[FN:M2] * (1.0 / LO_SCALE)          # even row tiles
        od = o[32 : 32 + FN] + o[32 + FN : 32 + M2] * (1.0 / LO_SCALE)
        ev = ev.reshape(FN, N_SLOT, RT)
        od = od.reshape(FN, N_SLOT, RT)
        o6r = o6.reshape(FN, N_RT, RT)
        o6r[:, 0::2, :] = ev
        o6r[:, 1::2, :] = od[:, : N_RT // 2, :]
        shards.append(o6[:, :ROWS_PER_CORE])
    full6 = np.concatenate(shards, axis=1)  # [6, N_PTS]
    out = np.ascontiguousarray(
        full6.reshape(B, 3, N_PTS).transpose(0, 2, 1)
    ).astype(np.float32)
    return out
